# revision 1
# baseline (speedup 1.0000x reference)
"""AlgebraicTransformerLM on 8 trn2 NeuronCores (Bass/Tile).

Sharding: DP=2 over batch (cores 0-3 = batch 0, cores 4-7 = batch 1) x TP=4
over heads / d_ffn / vocab inside each group. The residual stream lives
transposed on-device (x^T: [d, tok]); all weights are pre-transposed and
norm-weights/scales folded on the host. The rational softmax is computed via
ALiBi-augmented score matmuls (integer index rows + the same rounded slope on
both sides, so f32r rounding cannot cancel), an abs/recip/square pipeline on
ACT+DVE, denominator via a ones-column appended to V, and the denominator
reciprocal applied to the attention-output columns. Sublayer partial sums are
all-reduced in bf16 across each TP group of 4 cores.
"""
import contextlib
import math

import numpy as np

import concourse.bacc as bacc
import concourse.mybir as mybir
import concourse.tile as tile
from concourse.bass_utils import run_bass_kernel_spmd

F32 = mybir.dt.float32
F32R = mybir.dt.float32r
BF16 = mybir.dt.bfloat16

B, T, V, D, H, L = 2, 1024, 32000, 1024, 16, 4
DFF = 2730
DH = D // H
SCALE = 1.0 / math.sqrt(DH)
EPS = 1e-6

NCORES = 8
TP = 4
HPC = H // TP               # heads per core (4)
FSH = 2 * DH * HPC          # q+k rows per core (512)
VSH = DH * HPC              # v rows per core (256)
DFF_SH = 768                # padded DFF shard (4*768 >= 2730)
NFT_FF = DFF_SH // 128      # 6
VOC_SH = V // TP            # vocab shard per core (8000)
DT = D // 128               # 8
NSTRIP = T // 512           # 2
RG = [[0, 1, 2, 3], [4, 5, 6, 7]]
ALIBI = [2.0 ** (-8.0 * (i + 1) / H) for i in range(H)]

_CACHE = {}


class PoolSet:
    """Route scratch tags to per-bufs pools."""

    def __init__(self, scr, w4, m, row):
        self._map = {"scr": scr, "w4": w4, "nabs": m, "repsb": m,
                     "row": row, "rowr": row}

    _n = 0

    def tile(self, shape, dtype, tag):
        PoolSet._n += 1
        return self._map[tag].tile(shape, dtype, tag=tag, name=f"{tag}_{PoolSet._n}")


def _causal_tk(s):
    return list(range((s + 1) * (512 // 128)))


def _mask_base(tk, s):
    """affine_select base for tile (tk, strip s): keep where f + base - p >= 0,
    i.e. tq >= tk. None if the whole tile is causal-valid."""
    base = s * 512 - tk * 128
    return base if tk * 128 + 127 > s * 512 else None


def build_nc():
    nc = bacc.Bacc("TRN2", target_bir_lowering=False)

    x0T = nc.dram_tensor("x0T", [D, T], F32, kind="ExternalInput")
    qaug = nc.dram_tensor("qaug", [HPC, 2, T], F32, kind="ExternalInput")
    kaug = nc.dram_tensor("kaug", [HPC, 2, T], F32, kind="ExternalInput")
    wqkT = nc.dram_tensor("wqkT", [L, D, FSH], F32, kind="ExternalInput")
    wvT = nc.dram_tensor("wvT", [L, D, VSH], F32, kind="ExternalInput")
    woT = nc.dram_tensor("woT", [L, VSH, D], F32, kind="ExternalInput")
    wmT = nc.dram_tensor("wmT", [L, D, 2 * DFF_SH], F32, kind="ExternalInput")
    w3T = nc.dram_tensor("w3T", [L, DFF_SH, D], F32, kind="ExternalInput")
    membT = nc.dram_tensor("membT", [D, VOC_SH], F32, kind="ExternalInput")
    logits = nc.dram_tensor("logits", [T, VOC_SH], F32, kind="ExternalOutput")
    cc_in = [nc.dram_tensor(f"cc_in{i}", [D, T], BF16) for i in range(2 * L)]
    cc_out = [nc.dram_tensor(f"cc_out{i}", [D, T], BF16) for i in range(2 * L)]

    with tile.TileContext(nc) as tc, contextlib.ExitStack() as ctx:
        persist = ctx.enter_context(tc.tile_pool(name="persist", bufs=1))
        psA = ctx.enter_context(tc.tile_pool(name="psA", bufs=4, space="PSUM"))
        psAcc = ctx.enter_context(tc.tile_pool(name="psAcc", bufs=2, space="PSUM"))
        psSm = ctx.enter_context(tc.tile_pool(name="psSm", bufs=2, space="PSUM"))

        x = persist.tile([128, DT, T], F32, tag="x")
        nc.sync.dma_start(x[:], x0T[:].rearrange("(dt p) t -> p dt t", p=128))
        xn = persist.tile([128, DT, T], F32R, tag="xn")

        of = persist.tile([1, 128], F32, tag="ones_f")
        nc.vector.memset(of[:], 1.0)
        ones_st = persist.tile([1, 128], F32R, tag="ones_st")
        nc.vector.tensor_copy(ones_st[:], of[:])
        ocf = persist.tile([128, 1], F32, tag="ones_colf")
        nc.vector.memset(ocf[:], 1.0)
        ones_colb = persist.tile([128, 1], BF16, tag="ones_colb")
        nc.vector.tensor_copy(ones_colb[:], ocf[:])
        ones_b = persist.tile([128, 1], F32, tag="ones_bias")
        nc.vector.memset(ones_b[:], 1.0)

        def norm(spool):
            """xn = x / (mean|x| + eps), f32r (norm-w folded into weights)."""
            mags = []
            for s in range(NSTRIP):
                mags.append(psSm.tile([1, 512], F32, tag="small", name=f"mag{s}"))
            for dt in range(DT):
                xa = spool.tile([128, T], BF16, tag="nabs")
                nc.scalar.activation(xa[:], x[:, dt],
                                     mybir.ActivationFunctionType.Abs, scale=1.0)
                for s in range(NSTRIP):
                    nc.tensor.matmul(mags[s][:], ones_colb[:],
                                     xa[:, s * 512:(s + 1) * 512],
                                     start=(dt == 0), stop=(dt == DT - 1),
                                     skip_group_check=True)
            for s in range(NSTRIP):
                md = spool.tile([1, 512], F32, tag="row")
                nc.vector.tensor_scalar(md[:], mags[s][:], scalar1=1.0 / D,
                                        scalar2=EPS, op0=mybir.AluOpType.mult,
                                        op1=mybir.AluOpType.add)
                mr = spool.tile([1, 512], F32, tag="row")
                nc.vector.reciprocal_approx_fast(mr[:], md[:])
                mrr = spool.tile([1, 512], F32R, tag="rowr")
                nc.vector.tensor_copy(mrr[:], mr[:])
                rep = psA.tile([128, 512], F32, tag="ps")
                nc.tensor.matmul(rep[:], ones_st[:], mrr[:], start=True, stop=True)
                for dt in range(DT):
                    nc.vector.tensor_tensor(xn[:, dt, s * 512:(s + 1) * 512],
                                            x[:, dt, s * 512:(s + 1) * 512],
                                            rep[:], mybir.AluOpType.mult)

        def sigpipe(spool, s_ps, w4_out, mb):
            """w4 = (1 + s/(1+|s|))^4 from score psum [128, 512]."""
            a = spool.tile([128, 512], F32, tag="scr")
            nc.scalar.activation(a[:], s_ps[:],
                                 mybir.ActivationFunctionType.Abs, scale=1.0)
            d = spool.tile([128, 512], F32, tag="scr")
            nc.vector.tensor_scalar(d[:], a[:], scalar1=1.0, scalar2=None,
                                    op0=mybir.AluOpType.add,
                                    op1=mybir.AluOpType.bypass)
            r = spool.tile([128, 512], F32, tag="scr")
            nc.vector.reciprocal_approx_fast(r[:], d[:])
            u = spool.tile([128, 512], F32, tag="scr")
            nc.vector.tensor_tensor(u[:], s_ps[:], r[:], mybir.AluOpType.mult)
            w2 = spool.tile([128, 512], F32, tag="scr")
            nc.scalar.activation(w2[:], u[:], mybir.ActivationFunctionType.Square,
                                 bias=ones_b[:], scale=1.0)
            if mb is not None:
                w2m = spool.tile([128, 512], F32, tag="scr")
                nc.gpsimd.affine_select(w2m[:], w2[:], pattern=[[1, 512]], base=mb,
                                        channel_multiplier=-1,
                                        compare_op=mybir.AluOpType.is_ge, fill=0.0)
                w2 = w2m
            nc.scalar.activation(w4_out[:], w2[:],
                                 mybir.ActivationFunctionType.Square, scale=1.0)

        def attention(l, wpool, wvpool, apool, dpool, spool):
            norm(spool)
            # v projection (token-major) + ones column per head
            vaug = apool.tile([128, DT, HPC * 65], F32R, tag="vaug",
                              name=f"vaug{l}")
            for h in range(HPC):
                for tt in range(DT):
                    nc.vector.tensor_copy(vaug[:, tt, h * 65 + 64:h * 65 + 65],
                                          ocf[:])
            wv = wvpool.tile([128, DT, VSH], F32R, tag="wv", name=f"wv{l}")
            nc.sync.dma_start(
                wv[:], wvT[l].rearrange("(dt p) f -> p dt f", p=128).bitcast(F32R))
            for tt in range(DT):
                ps = psA.tile([128, 512], F32, tag="ps", name=f"vps{l}_{tt}")
                for dt in range(DT):
                    nc.tensor.matmul(ps[:, 0:VSH],
                                     xn[:, dt, tt * 128:(tt + 1) * 128],
                                     wv[:, dt], start=(dt == 0),
                                     stop=(dt == DT - 1))
                for h in range(HPC):
                    nc.vector.tensor_copy(
                        vaug[:, tt, h * 65:h * 65 + 64],
                        ps[:, h * 64:(h + 1) * 64])
            asb = apool.tile([128, 2, T], F32R, tag="asb", name=f"asb{l}")

            def project_pair(pair):
                """q then k rows for heads (2*pair, 2*pair+1) -> aug tiles."""
                qa, ka = [], []
                for h in (2 * pair, 2 * pair + 1):
                    qa.append(apool.tile([66, T], F32R, tag=f"qaug{h % 2}",
                                         name=f"qaug{h}_{l}"))
                    ka.append(apool.tile([66, T], F32R, tag=f"kaug{h % 2}",
                                         name=f"kaug{h}_{l}"))
                    nc.sync.dma_start(qa[-1][64:66, :], qaug[h].bitcast(F32R))
                    nc.sync.dma_start(ka[-1][64:66, :], kaug[h].bitcast(F32R))
                for qk, tgt in ((0, qa), (1, ka)):
                    ft = 2 * qk + pair        # col-tile in wqkT
                    wt = wpool.tile([128, DT, 128], F32R, tag="wt",
                                    name=f"wqk{l}_{ft}")
                    nc.sync.dma_start(
                        wt[:], wqkT[l][:, ft * 128:(ft + 1) * 128]
                        .rearrange("(dt p) f -> p dt f", p=128).bitcast(F32R))
                    for s in range(NSTRIP):
                        ps = psA.tile([128, 512], F32, tag="ps",
                                      name=f"qkps{l}_{ft}_{s}")
                        for dt in range(DT):
                            nc.tensor.matmul(ps[:], wt[:, dt],
                                             xn[:, dt, s * 512:(s + 1) * 512],
                                             start=(dt == 0), stop=(dt == DT - 1))
                        nc.vector.tensor_copy(tgt[0][0:64, s * 512:(s + 1) * 512],
                                              ps[0:64, :])
                        nc.vector.tensor_copy(tgt[1][0:64, s * 512:(s + 1) * 512],
                                              ps[64:128, :])
                return qa, ka

            def head_scores(h, qa_t, ka_t):
                for s in range(NSTRIP):
                    av = psAcc.tile([65, 512], F32, tag="av", name=f"av{l}_{h}_{s}")
                    tks = _causal_tk(s)
                    for i, tk in enumerate(tks):
                        sc = psA.tile([128, 512], F32, tag="ps",
                                      name=f"sc{l}_{h}_{s}_{tk}")
                        nc.tensor.matmul(sc[:], ka_t[:, tk * 128:(tk + 1) * 128],
                                         qa_t[:, s * 512:(s + 1) * 512],
                                         start=True, stop=True)
                        w4 = spool.tile([128, 512], F32R, tag="w4")
                        sigpipe(spool, sc, w4, _mask_base(tk, s))
                        nc.tensor.matmul(av[:], vaug[:, tk, h * 65:(h + 1) * 65],
                                         w4[:], start=(i == 0),
                                         stop=(i == len(tks) - 1),
                                         skip_group_check=True)
                    dd = spool.tile([1, 512], F32, tag="row")
                    nc.vector.tensor_scalar(dd[:], av[64:65, :], scalar1=16.0 * EPS,
                                            scalar2=None, op0=mybir.AluOpType.add,
                                            op1=mybir.AluOpType.bypass)
                    dr = spool.tile([1, 512], F32, tag="row")
                    nc.vector.reciprocal_approx_fast(dr[:], dd[:])
                    drr = spool.tile([1, 512], F32R, tag="rowr")
                    nc.vector.tensor_copy(drr[:], dr[:])
                    rep = psSm.tile([64, 512], F32, tag="small",
                                    name=f"rep{l}_{h}_{s}")
                    nc.tensor.matmul(rep[:], ones_st[:, 0:64], drr[:],
                                     start=True, stop=True)
                    reps = spool.tile([64, 512], F32, tag="repsb")
                    nc.scalar.activation(reps[:], rep[:],
                                         mybir.ActivationFunctionType.Copy,
                                         scale=1.0)
                    pair, half = h // 2, h % 2
                    nc.vector.tensor_tensor(
                        asb[64 * half:64 * (half + 1), pair, s * 512:(s + 1) * 512],
                        av[0:64, :], reps[:], mybir.AluOpType.mult)

            for pair in range(2):
                qa, ka = project_pair(pair)
                head_scores(2 * pair, qa[0], ka[0])
                head_scores(2 * pair + 1, qa[1], ka[1])
            # out projection
            dl = dpool.tile([128, DT, T], BF16, tag="delta", name=f"dla{l}")
            for ot in range(DT):
                wo = wpool.tile([128, 2, 128], F32R, tag="wo")
                nc.sync.dma_start(
                    wo[:], woT[l][:, ot * 128:(ot + 1) * 128]
                    .rearrange("(dt p) f -> p dt f", p=128).bitcast(F32R))
                for s in range(NSTRIP):
                    ps = psA.tile([128, 512], F32, tag="ps")
                    for dt in range(2):
                        nc.tensor.matmul(ps[:], wo[:, dt],
                                         asb[:, dt, s * 512:(s + 1) * 512],
                                         start=(dt == 0), stop=(dt == 1))
                    nc.vector.tensor_copy(dl[:, ot, s * 512:(s + 1) * 512], ps[:])
            return dl

        def swiglu(l, wpool, w3pool, apool, dpool, spool):
            norm(spool)
            hsb = apool.tile([128, NFT_FF, T], BF16, tag="hsb")
            for ft in range(NFT_FF):
                wg = wpool.tile([128, DT, 128], F32R, tag="wt")
                nc.sync.dma_start(
                    wg[:], wmT[l][:, ft * 128:(ft + 1) * 128]
                    .rearrange("(dt p) f -> p dt f", p=128).bitcast(F32R))
                wvv = wpool.tile([128, DT, 128], F32R, tag="wt")
                nc.sync.dma_start(
                    wvv[:], wmT[l][:, DFF_SH + ft * 128:DFF_SH + (ft + 1) * 128]
                    .rearrange("(dt p) f -> p dt f", p=128).bitcast(F32R))
                for s in range(NSTRIP):
                    gps = psA.tile([128, 512], F32, tag="ps")
                    vps = psA.tile([128, 512], F32, tag="ps")
                    for dt in range(DT):
                        nc.tensor.matmul(gps[:], wg[:, dt],
                                         xn[:, dt, s * 512:(s + 1) * 512],
                                         start=(dt == 0), stop=(dt == DT - 1))
                    for dt in range(DT):
                        nc.tensor.matmul(vps[:], wvv[:, dt],
                                         xn[:, dt, s * 512:(s + 1) * 512],
                                         start=(dt == 0), stop=(dt == DT - 1))
                    # h = (g^2*r + g) * v  (x0.5 folded into w3 host-side)
                    a = spool.tile([128, 512], F32, tag="scr")
                    nc.scalar.activation(a[:], gps[:],
                                         mybir.ActivationFunctionType.Abs, scale=1.0)
                    d = spool.tile([128, 512], F32, tag="scr")
                    nc.vector.tensor_scalar(d[:], a[:], scalar1=1.0, scalar2=None,
                                            op0=mybir.AluOpType.add,
                                            op1=mybir.AluOpType.bypass)
                    r = spool.tile([128, 512], F32, tag="scr")
                    nc.vector.reciprocal_approx_fast(r[:], d[:])
                    g2 = spool.tile([128, 512], F32, tag="scr")
                    nc.scalar.activation(g2[:], gps[:],
                                         mybir.ActivationFunctionType.Square,
                                         scale=1.0)
                    m1 = spool.tile([128, 512], F32, tag="scr")
                    nc.vector.tensor_tensor(m1[:], g2[:], r[:], mybir.AluOpType.mult)
                    m2 = spool.tile([128, 512], F32, tag="scr")
                    nc.vector.tensor_tensor(m2[:], m1[:], gps[:], mybir.AluOpType.add)
                    nc.vector.tensor_tensor(hsb[:, ft, s * 512:(s + 1) * 512],
                                            m2[:], vps[:], mybir.AluOpType.mult)
            w3sb = w3pool.tile([128, NFT_FF, D], BF16, tag="w3")
            nc.gpsimd.dma_start(
                w3sb[:], w3T[l].rearrange("(dt p) f -> p dt f", p=128))
            dl = dpool.tile([128, DT, T], BF16, tag="delta", name=f"dlm{l}")
            for ot in range(DT):
                for s in range(NSTRIP):
                    ps = psA.tile([128, 512], F32, tag="ps")
                    for ft in range(NFT_FF):
                        nc.tensor.matmul(ps[:], w3sb[:, ft, ot * 128:(ot + 1) * 128],
                                         hsb[:, ft, s * 512:(s + 1) * 512],
                                         start=(ft == 0), stop=(ft == NFT_FF - 1))
                    nc.vector.tensor_copy(dl[:, ot, s * 512:(s + 1) * 512], ps[:])
            return dl

        def allreduce_add(dpool, dl, idx):
            nc.sync.dma_start(
                cc_in[idx][:].rearrange("(dt p) t -> p dt t", p=128), dl[:])
            nc.gpsimd.collective_compute(
                "AllReduce", mybir.AluOpType.add,
                ins=[cc_in[idx][:]], outs=[cc_out[idx][:]], replica_groups=RG)
            db = dpool.tile([128, DT, T], BF16, tag="delta", name=f"db{idx}")
            nc.sync.dma_start(
                db[:], cc_out[idx][:].rearrange("(dt p) t -> p dt t", p=128))
            for dt in range(DT):
                nc.vector.tensor_tensor(x[:, dt], x[:, dt], db[:, dt],
                                        mybir.AluOpType.add)

        with tc.tile_pool(name="wpool", bufs=3) as wpool, \
             tc.tile_pool(name="wvpool", bufs=1) as wvpool, \
             tc.tile_pool(name="w3pool", bufs=1) as w3pool, \
             tc.tile_pool(name="apool", bufs=1) as apool, \
             tc.tile_pool(name="dpool", bufs=2) as dpool, \
             tc.tile_pool(name="scrpool", bufs=5) as scrpool, \
             tc.tile_pool(name="w4pool", bufs=2) as w4pool, \
             tc.tile_pool(name="mpool", bufs=2) as mpool, \
             tc.tile_pool(name="rowpool", bufs=2) as rowpool:
            spool = PoolSet(scrpool, w4pool, mpool, rowpool)
            for l in range(L):
                dl = attention(l, wpool, wvpool, apool, dpool, spool)
                allreduce_add(dpool, dl, 2 * l)
                dl = swiglu(l, wpool, w3pool, apool, dpool, spool)
                allreduce_add(dpool, dl, 2 * l + 1)
            norm(spool)

        with tc.tile_pool(name="lmw", bufs=2) as lmw, \
             tc.tile_pool(name="lms", bufs=4) as lms:
            nvs = (VOC_SH + 511) // 512
            for vs in range(nvs):
                vw = min(512, VOC_SH - vs * 512)
                wt = lmw.tile([128, DT, 512], F32R, tag="wemb")
                nc.sync.dma_start(
                    wt[:, :, :vw], membT[:, vs * 512:vs * 512 + vw]
                    .rearrange("(dt p) f -> p dt f", p=128).bitcast(F32R))
                for tt in range(DT):
                    ps = psA.tile([128, 512], F32, tag="ps")
                    for dt in range(DT):
                        nc.tensor.matmul(ps[:, :vw],
                                         xn[:, dt, tt * 128:(tt + 1) * 128],
                                         wt[:, dt, :vw],
                                         start=(dt == 0), stop=(dt == DT - 1))
                    ls = lms.tile([128, 512], F32, tag="lmsb")
                    nc.vector.tensor_copy(ls[:, :vw], ps[:, :vw])
                    nc.sync.dma_start(
                        logits[tt * 128:(tt + 1) * 128, vs * 512:vs * 512 + vw],
                        ls[:, :vw])
    nc.compile()
    return nc


def _prep_inputs(input_ids, emb, qkv_w, out_w, n1_w, n2_w, wm_w, w3_w, fn_w):
    ids = np.asarray(input_ids)
    emb = np.asarray(emb, dtype=np.float32)
    x0 = emb[ids]                                   # [B, T, D]
    iota = np.arange(T, dtype=np.float32)
    qkv_w = np.asarray(qkv_w, dtype=np.float32)
    out_w = np.asarray(out_w, dtype=np.float32)
    wm_w = np.asarray(wm_w, dtype=np.float32)
    w3_w = np.asarray(w3_w, dtype=np.float32)
    n1_w = np.asarray(n1_w, dtype=np.float32)
    n2_w = np.asarray(n2_w, dtype=np.float32)
    fn_w = np.asarray(fn_w, dtype=np.float32)
    per_core = []
    for c in range(NCORES):
        b, r = c // TP, c % TP
        heads = list(range(HPC * r, HPC * r + HPC))
        qa = np.stack([np.stack([-iota, np.full(T, ALIBI[h], np.float32)])
                       for h in heads]).astype(np.float32)
        ka = np.stack([np.stack([np.full(T, ALIBI[h], np.float32), iota])
                       for h in heads]).astype(np.float32)
        wqk = np.empty((L, D, FSH), np.float32)
        wv = np.empty((L, D, VSH), np.float32)
        wo = np.empty((L, VSH, D), np.float32)
        wm = np.zeros((L, D, 2 * DFF_SH), np.float32)
        w3 = np.zeros((L, DFF_SH, D), np.float32)
        for l in range(L):
            q3 = qkv_w[l].reshape(3, H, DH, D)
            qrows = q3[0, heads].reshape(VSH, D) * SCALE
            krows = q3[1, heads].reshape(VSH, D)
            vrows = q3[2, heads].reshape(VSH, D)
            n1 = n1_w[l][:, None]                   # fold into d-rows of W^T
            wqk[l] = np.concatenate([qrows, krows], 0).T * n1
            wv[l] = vrows.T * n1
            ow = out_w[l].reshape(D, H, DH)[:, heads].reshape(D, VSH)
            wo[l] = ow.T
            n2 = n2_w[l][:, None]
            g0, g1 = DFF_SH * r, min(DFF_SH * (r + 1), DFF)
            ng = g1 - g0
            if ng > 0:
                wm[l, :, :ng] = wm_w[l][g0:g1].T * n2
                wm[l, :, DFF_SH:DFF_SH + ng] = wm_w[l][DFF + g0:DFF + g1].T * n2
                w3[l, :ng] = 0.5 * w3_w[l][:, g0:g1].T
        memb = (emb[VOC_SH * r:VOC_SH * (r + 1)] * fn_w[None, :]).T
        per_core.append(dict(
            x0T=np.ascontiguousarray(x0[b].T),
            qaug=qa, kaug=ka,
            wqkT=np.ascontiguousarray(wqk), wvT=np.ascontiguousarray(wv),
            woT=np.ascontiguousarray(wo), wmT=np.ascontiguousarray(wm),
            w3T=np.ascontiguousarray(w3), membT=np.ascontiguousarray(memb),
        ))
    return per_core


def kernel(**inputs):
    if "nc" not in _CACHE:
        _CACHE["nc"] = build_nc()
    nc = _CACHE["nc"]
    per_core = _prep_inputs(**inputs)
    res = run_bass_kernel_spmd(nc, per_core, core_ids=list(range(NCORES)),
                               **_CACHE.get("run_kwargs", {}))
    _CACHE["last_result"] = res
    out = np.empty((B, T, V), np.float32)
    for c in range(NCORES):
        b, r = c // TP, c % TP
        out[b, :, VOC_SH * r:VOC_SH * (r + 1)] = res.results[c]["logits"]
    return out



# revision 29
# speedup vs baseline: 1.1996x; 1.1996x over previous
"""AlgebraicTransformerLM on 8 trn2 NeuronCores (Bass/Tile), v2.

Sharding: DP=2 over batch x TP=4 over heads / d_ffn / vocab (cores 0-3 =
batch 0, 4-7 = batch 1). vs v1:
  - bf16 weights + bf16 normed activations (xn) for all projections; the
    ALiBi-augmented score matmuls stay f32r (iota rows need f32 range).
  - The sublayer AllReduce is chunked by 512-token strip and fired as soon
    as that strip's output projection lands, so the collective flies while
    the other strip / next sublayer computes (keeps the PE HAM-warm).
  - Rational-sigmoid pipeline: one PSUM read (ACT copy -> bf16), fused
    |s|+1 via tensor_scalar(abs_max,add), divide (or recip+mul fallback),
    squares split ACT/DVE, causal mask via affine_select on POOL.
  - Residual adds / xn scaling split across DVE and POOL.
Host prep: fold norm weights into following matmuls, precompute xn0 (norm
of embedding output) so layer 0 starts without a device norm.
"""
import contextlib
import math

import numpy as np

import concourse.bacc as bacc
import concourse.mybir as mybir
import concourse.tile as tile
from concourse.bass_utils import run_bass_kernel_spmd

F32 = mybir.dt.float32
F32R = mybir.dt.float32r
FP16 = mybir.dt.float16

B, T, V, D, H, L = 2, 1024, 32000, 1024, 16, 4
DFF = 2730
DH = D // H
SCALE = 1.0 / math.sqrt(DH)
EPS = 1e-6

NCORES = 8
TP = 4
HPC = H // TP               # heads per core (4)
FSH = 2 * DH * HPC          # q+k rows per core (512)
VSH = DH * HPC              # v rows per core (256)
DFF_SH = 768                # padded DFF shard (4*768 >= 2730)
NFT_FF = DFF_SH // 128      # 6
VOC_SH = V // TP            # vocab shard per core (8000)
DT = D // 128               # 8
NSTRIP = T // 512           # 2
RG = [[0, 1, 2, 3], [4, 5, 6, 7]]
ALIBI = [2.0 ** (-8.0 * (i + 1) / H) for i in range(H)]

_CACHE = {}

AF = mybir.ActivationFunctionType
ALU = mybir.AluOpType


class PoolSet:
    """Route scratch tags to per-bufs pools."""

    def __init__(self, **pools):
        self._map = pools

    _n = 0

    def tile(self, shape, dtype, tag):
        PoolSet._n += 1
        return self._map[tag].tile(shape, dtype, tag=tag, name=f"{tag}_{PoolSet._n}")


def _causal_tk(s):
    return list(range((s + 1) * (512 // 128)))


def _mask_base(tk, s):
    """affine_select base for tile (tk, strip s): keep where f + base - p >= 0,
    i.e. tq >= tk. None if the whole tile is causal-valid."""
    base = s * 512 - tk * 128
    return base if tk * 128 + 127 > s * 512 else None


def build_nc(use_divide=True, use_accum_dma=True):
    nc = bacc.Bacc("TRN2", target_bir_lowering=False)

    x0T = nc.dram_tensor("x0T", [D, T], F32, kind="ExternalInput")
    xn0T = nc.dram_tensor("xn0T", [D, T], FP16, kind="ExternalInput")
    qaug = nc.dram_tensor("qaug", [HPC, 2, T], F32, kind="ExternalInput")
    kaug = nc.dram_tensor("kaug", [HPC, 2, T], F32, kind="ExternalInput")
    wqkT = nc.dram_tensor("wqkT", [L, D, FSH], FP16, kind="ExternalInput")
    wvT = nc.dram_tensor("wvT", [L, D, VSH], FP16, kind="ExternalInput")
    woT = nc.dram_tensor("woT", [L, VSH, D], FP16, kind="ExternalInput")
    wmT = nc.dram_tensor("wmT", [L, D, 2 * DFF_SH], FP16, kind="ExternalInput")
    w3T = nc.dram_tensor("w3T", [L, DFF_SH, D], FP16, kind="ExternalInput")
    membT = nc.dram_tensor("membT", [D, VOC_SH], FP16, kind="ExternalInput")
    logits = nc.dram_tensor("logits", [T, VOC_SH], F32, kind="ExternalOutput")
    NCH = 2 * L * NSTRIP
    cc_in = [nc.dram_tensor(f"cc_in{i}", [D, 512], FP16) for i in range(NCH)]
    cc_out = [nc.dram_tensor(f"cc_out{i}", [D, 512], FP16) for i in range(NCH)]

    with tile.TileContext(nc) as tc, contextlib.ExitStack() as ctx:
        persist = ctx.enter_context(tc.tile_pool(name="persist", bufs=1))
        psA = ctx.enter_context(tc.tile_pool(name="psA", bufs=4, space="PSUM"))
        psAcc = ctx.enter_context(tc.tile_pool(name="psAcc", bufs=2, space="PSUM"))
        psSm = ctx.enter_context(tc.tile_pool(name="psSm", bufs=1, space="PSUM"))

        x = persist.tile([128, DT, T], F32, tag="x")
        nc.sync.dma_start(x[:], x0T[:].rearrange("(dt p) t -> p dt t", p=128))
        xn = persist.tile([128, DT, T], FP16, tag="xn")
        nc.sync.dma_start(xn[:], xn0T[:].rearrange("(dt p) t -> p dt t", p=128))

        ocf = persist.tile([128, 1], F32, tag="ones_colf")
        nc.vector.memset(ocf[:], 1.0)
        ones_colb = persist.tile([128, 1], FP16, tag="ones_colb")
        nc.vector.tensor_copy(ones_colb[:], ocf[:])
        orf = persist.tile([1, 128], F32, tag="ones_rowf")
        nc.vector.memset(orf[:], 1.0)
        ones_rowb = persist.tile([1, 128], FP16, tag="ones_rowb")
        nc.vector.tensor_copy(ones_rowb[:], orf[:])
        ones_b = persist.tile([128, 1], F32, tag="ones_bias")
        nc.vector.memset(ones_b[:], 1.0)

        def sigrecip(spool, src_ps, on_act):
            """r = 1/(1+|s|) from a [128, 512] f32 PSUM tile. ACT variant:
            exp(-ln(1+|s|)) via LUT; DVE variant: reciprocal_approx_fast."""
            if on_act:
                a = spool.tile([128, 512], FP16, tag="aa")
                nc.scalar.activation(a[:], src_ps[:], AF.Abs, scale=1.0)
                ln = spool.tile([128, 512], FP16, tag="rr")
                nc.scalar.activation(ln[:], a[:], AF.Ln, bias=1.0, scale=1.0)
                r = spool.tile([128, 512], FP16, tag="aa")
                nc.scalar.activation(r[:], ln[:], AF.Exp, scale=-1.0)
            else:
                a = spool.tile([128, 512], F32, tag="af")
                nc.scalar.activation(a[:], src_ps[:], AF.Abs, scale=1.0)
                d = spool.tile([128, 512], F32, tag="af")
                nc.vector.tensor_scalar(d[:], a[:], scalar1=1.0, scalar2=None,
                                        op0=ALU.add, op1=ALU.bypass)
                r = spool.tile([128, 512], F32, tag="af")
                nc.vector.reciprocal_approx_fast(r[:], d[:])
            return r

        def sigpipe(spool, s_ps, w4_out, mb, on_act):
            """w4 = (1 + s/(1+|s|))^4 in bf16 from score psum [128, 512]."""
            r = sigrecip(spool, s_ps, on_act)
            u = spool.tile([128, 512], FP16, tag="uu")
            nc.vector.tensor_tensor(u[:], s_ps[:], r[:], ALU.mult)
            w2 = spool.tile([128, 512], FP16, tag="w2")
            nc.scalar.activation(w2[:], u[:], AF.Square, bias=ones_b[:],
                                 scale=1.0)
            if mb is not None:
                w2m = spool.tile([128, 512], FP16, tag="w2m")
                nc.gpsimd.affine_select(w2m[:], w2[:], pattern=[[1, 512]],
                                        base=mb, channel_multiplier=-1,
                                        compare_op=ALU.is_ge, fill=0.0)
                w2 = w2m
            nc.vector.tensor_tensor(w4_out[:], w2[:], w2[:], ALU.mult)

        def addnorm(s, idx, spool, dpool):
            """x[:, :, strip] += AR-chunk idx; xn strip = normed x (bf16)."""
            sl = slice(s * 512, (s + 1) * 512)
            if use_accum_dma:
                nc.gpsimd.dma_start(
                    x[:, :, sl],
                    cc_out[idx][:].rearrange("(dt p) t -> p dt t", p=128),
                    accum_op=ALU.add)
            else:
                db = dpool.tile([128, DT, 512], FP16, tag="db")
                nc.sync.dma_start(db[:],
                                  cc_out[idx][:].rearrange(
                                      "(dt p) t -> p dt t", p=128))
                for dt in range(DT):
                    eng = nc.vector if dt < 4 else nc.gpsimd
                    eng.tensor_tensor(x[:, dt, sl], x[:, dt, sl], db[:, dt],
                                      ALU.add)
            xa = spool.tile([128, DT, 512], FP16, tag="nabs")
            nc.scalar.activation(xa[:], x[:, :, sl], AF.Abs, scale=1.0)
            mag = psSm.tile([1, 512], F32, tag="small", name=f"mag{idx}")
            for dt in range(DT):
                nc.tensor.matmul(mag[:], ones_colb[:], xa[:, dt],
                                 start=(dt == 0), stop=(dt == DT - 1),
                                 skip_group_check=True)
            md = spool.tile([1, 512], F32, tag="row")
            nc.vector.tensor_scalar(md[:], mag[:], scalar1=1.0 / D,
                                    scalar2=EPS, op0=ALU.mult, op1=ALU.add)
            mr = spool.tile([1, 512], F32, tag="row")
            nc.vector.reciprocal_approx_fast(mr[:], md[:])
            mrb = spool.tile([1, 512], FP16, tag="rowb")
            nc.vector.tensor_copy(mrb[:], mr[:])
            rep = psA.tile([128, 512], F32, tag="ps", name=f"rep{idx}")
            nc.tensor.matmul(rep[:], ones_rowb[:], mrb[:], start=True,
                             stop=True)
            repb = spool.tile([128, 512], FP16, tag="repb")
            nc.scalar.activation(repb[:], rep[:], AF.Copy, scale=1.0)
            for dt in range(DT):
                eng = nc.gpsimd if dt < 4 else nc.vector
                eng.tensor_tensor(xn[:, dt, sl], x[:, dt, sl], repb[:],
                                  ALU.mult)

        def attention(l, wpool, apool, dpool, spool, pending):
            # whole-layer weight loads (bf16)
            wqk = wpool.tile([128, DT, FSH], FP16, tag="wqk")
            nc.sync.dma_start(wqk[:],
                              wqkT[l].rearrange("(dt p) f -> p dt f", p=128))
            wv = wpool.tile([128, DT, VSH], FP16, tag="wv")
            nc.sync.dma_start(wv[:],
                              wvT[l].rearrange("(dt p) f -> p dt f", p=128))
            wo = wpool.tile([128, 2, D], FP16, tag="wo")
            nc.sync.dma_start(wo[:],
                              woT[l].rearrange("(pp p) f -> p pp f", p=128))

            vaug = apool.tile([128, DT, HPC * 65], FP16, tag="vaug",
                              name=f"vaug{l}")
            qa = [apool.tile([66, T], F32R, tag=f"qa{h}", name=f"qa{h}_{l}")
                  for h in range(HPC)]
            ka = [apool.tile([66, T], F32R, tag=f"ka{h}", name=f"ka{h}_{l}")
                  for h in range(HPC)]
            for h in range(HPC):
                nc.sync.dma_start(qa[h][64:66, :], qaug[h].bitcast(F32R))
                nc.sync.dma_start(ka[h][64:66, :], kaug[h].bitcast(F32R))
            asb = apool.tile([128, 2, T], FP16, tag="asb", name=f"asb{l}")

            def vproj(s):
                for tt in range(s * 4, s * 4 + 4):
                    ps = psA.tile([128, 512], F32, tag="ps",
                                  name=f"vps{l}_{tt}")
                    for dt in range(DT):
                        nc.tensor.matmul(ps[:, 0:VSH],
                                         xn[:, dt, tt * 128:(tt + 1) * 128],
                                         wv[:, dt], start=(dt == 0),
                                         stop=(dt == DT - 1))
                    for h in range(HPC):
                        nc.vector.tensor_copy(
                            vaug[:, tt, h * 65:h * 65 + 64],
                            ps[:, h * 64:(h + 1) * 64])
                        nc.vector.memset(vaug[:, tt, h * 65 + 64:h * 65 + 65],
                                         1.0)

            def qkproj(s):
                sl = slice(s * 512, (s + 1) * 512)
                for ft in range(4):
                    qk, pair = ft // 2, ft % 2
                    ps = psA.tile([128, 512], F32, tag="ps",
                                  name=f"qkps{l}_{ft}_{s}")
                    for dt in range(DT):
                        nc.tensor.matmul(ps[:],
                                         wqk[:, dt, ft * 128:(ft + 1) * 128],
                                         xn[:, dt, sl], start=(dt == 0),
                                         stop=(dt == DT - 1))
                    tgt = qa if qk == 0 else ka
                    nc.scalar.activation(tgt[2 * pair][0:64, sl], ps[0:64, :],
                                         AF.Copy, scale=1.0)
                    nc.scalar.activation(tgt[2 * pair + 1][0:64, sl],
                                         ps[64:128, :], AF.Copy, scale=1.0)

            def head_scores(h, s):
                av = psAcc.tile([65, 512], F32, tag="av", name=f"av{l}_{h}_{s}")
                tks = _causal_tk(s)
                for i, tk in enumerate(tks):
                    sc = psA.tile([128, 512], F32, tag="ps",
                                  name=f"sc{l}_{h}_{s}_{tk}")
                    nc.tensor.matmul(sc[:], ka[h][:, tk * 128:(tk + 1) * 128],
                                     qa[h][:, s * 512:(s + 1) * 512],
                                     start=True, stop=True)
                    w4 = spool.tile([128, 512], FP16, tag="w4")
                    sigpipe(spool, sc, w4, _mask_base(tk, s),
                            on_act=(i % 2 == 0))
                    nc.tensor.matmul(av[:], vaug[:, tk, h * 65:(h + 1) * 65],
                                     w4[:], start=(i == 0),
                                     stop=(i == len(tks) - 1),
                                     skip_group_check=True)
                dd = spool.tile([1, 512], F32, tag="row")
                nc.vector.tensor_scalar(dd[:], av[64:65, :],
                                        scalar1=16.0 * EPS, scalar2=None,
                                        op0=ALU.add, op1=ALU.bypass)
                dr = spool.tile([1, 512], F32, tag="row")
                nc.vector.reciprocal_approx_fast(dr[:], dd[:])
                drb = spool.tile([1, 512], FP16, tag="rowb")
                nc.vector.tensor_copy(drb[:], dr[:])
                rep = psSm.tile([64, 512], F32, tag="rep64",
                                name=f"rep{l}_{h}_{s}")
                nc.tensor.matmul(rep[:], ones_rowb[:, 0:64], drb[:],
                                 start=True, stop=True)
                reps = spool.tile([64, 512], FP16, tag="repsb")
                nc.scalar.activation(reps[:], rep[:], AF.Copy, scale=1.0)
                pair, half = h // 2, h % 2
                nc.vector.tensor_tensor(
                    asb[64 * half:64 * (half + 1), pair,
                        s * 512:(s + 1) * 512],
                    av[0:64, :], reps[:], ALU.mult)

            def outproj(s):
                sl = slice(s * 512, (s + 1) * 512)
                dls = dpool.tile([128, DT, 512], FP16, tag="dls",
                                 name=f"dla{l}_{s}")
                for ot in range(DT):
                    ps = psA.tile([128, 512], F32, tag="ps")
                    for p in range(2):
                        nc.tensor.matmul(ps[:],
                                         wo[:, p, ot * 128:(ot + 1) * 128],
                                         asb[:, p, sl], start=(p == 0),
                                         stop=(p == 1))
                    if ot % 2 == 0:
                        nc.scalar.activation(dls[:, ot], ps[:], AF.Copy,
                                             scale=1.0)
                    else:
                        nc.vector.tensor_copy(dls[:, ot], ps[:])
                return dls

            def fire(dls, idx):
                nc.sync.dma_start(
                    cc_in[idx][:].rearrange("(dt p) t -> p dt t", p=128),
                    dls[:])
                nc.gpsimd.collective_compute(
                    "AllReduce", ALU.add, ins=[cc_in[idx][:]],
                    outs=[cc_out[idx][:]], replica_groups=RG)

            idx0 = (2 * l) * NSTRIP
            # strip 0
            vproj(0)
            qkproj(0)
            for h in range(HPC):
                head_scores(h, 0)
            dls = outproj(0)
            fire(dls, idx0)
            # deferred addnorm from previous sublayer (strip 1)
            for fn in pending:
                fn()
            # strip 1
            vproj(1)
            qkproj(1)
            for h in range(HPC):
                head_scores(h, 1)
            dls = outproj(1)
            fire(dls, idx0 + 1)

        def swiglu(l, wpool, wspool, apool, dpool, spool, pending):
            w3sb = wpool.tile([128, NFT_FF, D], FP16, tag="w3")
            nc.sync.dma_start(w3sb[:],
                              w3T[l].rearrange("(ft p) f -> p ft f", p=128))

            def strip(s, idx):
                hsb = dpool.tile([128, NFT_FF, 512], FP16, tag="hsb",
                                 name=f"hsb{l}_{s}")
                sl = slice(s * 512, (s + 1) * 512)
                for ft in range(NFT_FF):
                    wmg = wspool.tile([128, DT, 128], FP16, tag="wmg")
                    nc.sync.dma_start(
                        wmg[:], wmT[l][:, ft * 128:(ft + 1) * 128]
                        .rearrange("(dt p) f -> p dt f", p=128))
                    wmv = wspool.tile([128, DT, 128], FP16, tag="wmv")
                    nc.sync.dma_start(
                        wmv[:],
                        wmT[l][:, DFF_SH + ft * 128:DFF_SH + (ft + 1) * 128]
                        .rearrange("(dt p) f -> p dt f", p=128))
                    gps = psA.tile([128, 512], F32, tag="ps")
                    vps = psA.tile([128, 512], F32, tag="ps")
                    for dt in range(DT):
                        nc.tensor.matmul(gps[:], wmg[:, dt],
                                         xn[:, dt, sl], start=(dt == 0),
                                         stop=(dt == DT - 1))
                    for dt in range(DT):
                        nc.tensor.matmul(vps[:], wmv[:, dt],
                                         xn[:, dt, sl], start=(dt == 0),
                                         stop=(dt == DT - 1))
                    # h = g*(1+u)*v with u = g/(1+|g|)  (x0.5 folded into w3)
                    r = sigrecip(spool, gps, on_act=True)
                    gb = spool.tile([128, 512], FP16, tag="gb")
                    nc.vector.tensor_copy(gb[:], gps[:])
                    u = spool.tile([128, 512], FP16, tag="uu")
                    nc.vector.tensor_tensor(u[:], gb[:], r[:], ALU.mult)
                    t = spool.tile([128, 512], FP16, tag="tt")
                    nc.scalar.activation(t[:], u[:], AF.Copy, bias=1.0,
                                         scale=1.0)
                    m2 = spool.tile([128, 512], FP16, tag="m2")
                    nc.gpsimd.tensor_tensor(m2[:], gb[:], t[:], ALU.mult)
                    nc.vector.tensor_tensor(hsb[:, ft], m2[:], vps[:],
                                            ALU.mult)
                dls = dpool.tile([128, DT, 512], FP16, tag="dls",
                                 name=f"dlm{l}_{s}")
                for ot in range(DT):
                    ps = psA.tile([128, 512], F32, tag="ps")
                    for ft in range(NFT_FF):
                        nc.tensor.matmul(ps[:],
                                         w3sb[:, ft, ot * 128:(ot + 1) * 128],
                                         hsb[:, ft], start=(ft == 0),
                                         stop=(ft == NFT_FF - 1))
                    if ot % 2 == 0:
                        nc.scalar.activation(dls[:, ot], ps[:], AF.Copy,
                                             scale=1.0)
                    else:
                        nc.vector.tensor_copy(dls[:, ot], ps[:])
                nc.sync.dma_start(
                    cc_in[idx][:].rearrange("(dt p) t -> p dt t", p=128),
                    dls[:])
                nc.gpsimd.collective_compute(
                    "AllReduce", ALU.add, ins=[cc_in[idx][:]],
                    outs=[cc_out[idx][:]], replica_groups=RG)

            idx0 = (2 * l + 1) * NSTRIP
            strip(0, idx0)
            for fn in pending:
                fn()
            strip(1, idx0 + 1)

        with tc.tile_pool(name="wpool", bufs=1) as wpool, \
             tc.tile_pool(name="wspool", bufs=3) as wspool, \
             tc.tile_pool(name="apool", bufs=1) as apool, \
             tc.tile_pool(name="dpool", bufs=2) as dpool, \
             tc.tile_pool(name="scrpool", bufs=3) as scrpool, \
             tc.tile_pool(name="ewpool", bufs=2) as ewpool, \
             tc.tile_pool(name="w4pool", bufs=2) as w4pool, \
             tc.tile_pool(name="npool", bufs=1) as npool, \
             tc.tile_pool(name="mpool", bufs=2) as mpool, \
             tc.tile_pool(name="rowpool", bufs=2) as rowpool:
            spool = PoolSet(aa=scrpool, rr=scrpool, uu=scrpool, af=scrpool,
                            w2=scrpool, w2m=scrpool, gb=ewpool, tt=ewpool,
                            m2=ewpool, w4=w4pool, nabs=npool, repb=mpool,
                            repsb=mpool, row=rowpool, rowb=rowpool)

            def an(s, idx):
                return lambda: addnorm(s, idx, spool, dpool)

            # AR chunk ids per layer: 4l (att s0), 4l+1 (att s1),
            # 4l+2 (ffn s0), 4l+3 (ffn s1). Each addnorm is emitted as late
            # as possible so the collective flies under compute: att-s1
            # addnorm lands between the ffn strips, ffn-s1 addnorm between
            # the next layer's attention strips.
            pend = []
            for l in range(L):
                attention(l, wpool, apool, dpool, spool, pend)
                addnorm(0, 4 * l, spool, dpool)
                swiglu(l, wpool, wspool, apool, dpool, spool,
                       [an(1, 4 * l + 1)])
                addnorm(0, 4 * l + 2, spool, dpool)
                pend = [an(1, 4 * l + 3)]
            for fn in pend:
                fn()

        with tc.tile_pool(name="lmw", bufs=2) as lmw, \
             tc.tile_pool(name="lms", bufs=4) as lms:
            nvs = (VOC_SH + 511) // 512
            for vs in range(nvs):
                vw = min(512, VOC_SH - vs * 512)
                wt = lmw.tile([128, DT, 512], FP16, tag="wemb")
                nc.sync.dma_start(
                    wt[:, :, :vw], membT[:, vs * 512:vs * 512 + vw]
                    .rearrange("(dt p) f -> p dt f", p=128))
                for tt in range(DT):
                    ps = psA.tile([128, 512], F32, tag="ps")
                    for dt in range(DT):
                        nc.tensor.matmul(ps[:, :vw],
                                         xn[:, dt, tt * 128:(tt + 1) * 128],
                                         wt[:, dt, :vw],
                                         start=(dt == 0), stop=(dt == DT - 1))
                    ls = lms.tile([128, 512], F32, tag="lmsb")
                    if tt % 2 == 0:
                        nc.scalar.activation(ls[:, :vw], ps[:, :vw], AF.Copy,
                                             scale=1.0)
                    else:
                        nc.vector.tensor_copy(ls[:, :vw], ps[:, :vw])
                    nc.sync.dma_start(
                        logits[tt * 128:(tt + 1) * 128,
                               vs * 512:vs * 512 + vw],
                        ls[:, :vw])
    nc.compile()
    return nc


def _prep_inputs(input_ids, emb, qkv_w, out_w, n1_w, n2_w, wm_w, w3_w, fn_w):
    ids = np.asarray(input_ids)
    emb = np.asarray(emb, dtype=np.float32)
    x0 = emb[ids]                                   # [B, T, D]
    mag = np.mean(np.abs(x0), axis=-1, keepdims=True)
    xn0 = x0 / (mag + EPS)
    iota = np.arange(T, dtype=np.float32)
    qkv_w = np.asarray(qkv_w, dtype=np.float32)
    out_w = np.asarray(out_w, dtype=np.float32)
    wm_w = np.asarray(wm_w, dtype=np.float32)
    w3_w = np.asarray(w3_w, dtype=np.float32)
    n1_w = np.asarray(n1_w, dtype=np.float32)
    n2_w = np.asarray(n2_w, dtype=np.float32)
    fn_w = np.asarray(fn_w, dtype=np.float32)
    per_core = []
    for c in range(NCORES):
        b, r = c // TP, c % TP
        heads = list(range(HPC * r, HPC * r + HPC))
        qa = np.stack([np.stack([-iota, np.full(T, ALIBI[h], np.float32)])
                       for h in heads]).astype(np.float32)
        ka = np.stack([np.stack([np.full(T, ALIBI[h], np.float32), iota])
                       for h in heads]).astype(np.float32)
        wqk = np.empty((L, D, FSH), np.float32)
        wv = np.empty((L, D, VSH), np.float32)
        wo = np.empty((L, VSH, D), np.float32)
        wm = np.zeros((L, D, 2 * DFF_SH), np.float32)
        w3 = np.zeros((L, DFF_SH, D), np.float32)
        for l in range(L):
            q3 = qkv_w[l].reshape(3, H, DH, D)
            qrows = q3[0, heads].reshape(VSH, D) * SCALE
            krows = q3[1, heads].reshape(VSH, D)
            vrows = q3[2, heads].reshape(VSH, D)
            n1 = n1_w[l][:, None]                   # fold into d-rows of W^T
            wqk[l] = np.concatenate([qrows, krows], 0).T * n1
            wv[l] = vrows.T * n1
            ow = out_w[l].reshape(D, H, DH)[:, heads].reshape(D, VSH)
            wo[l] = ow.T
            n2 = n2_w[l][:, None]
            g0, g1 = DFF_SH * r, min(DFF_SH * (r + 1), DFF)
            ng = g1 - g0
            if ng > 0:
                wm[l, :, :ng] = wm_w[l][g0:g1].T * n2
                wm[l, :, DFF_SH:DFF_SH + ng] = wm_w[l][DFF + g0:DFF + g1].T * n2
                w3[l, :ng] = 0.5 * w3_w[l][:, g0:g1].T
        memb = (emb[VOC_SH * r:VOC_SH * (r + 1)] * fn_w[None, :]).T
        per_core.append(dict(
            x0T=np.ascontiguousarray(x0[b].T),
            xn0T=np.ascontiguousarray(xn0[b].T).astype(np.float16),
            qaug=qa, kaug=ka,
            wqkT=np.ascontiguousarray(wqk).astype(np.float16),
            wvT=np.ascontiguousarray(wv).astype(np.float16),
            woT=np.ascontiguousarray(wo).astype(np.float16),
            wmT=np.ascontiguousarray(wm).astype(np.float16),
            w3T=np.ascontiguousarray(w3).astype(np.float16),
            membT=np.ascontiguousarray(memb).astype(np.float16),
        ))
    return per_core


def kernel(**inputs):
    if "nc" not in _CACHE:
        try:
            _CACHE["nc"] = build_nc(use_divide=True)
        except Exception:
            _CACHE["nc"] = build_nc(use_divide=False)
    nc = _CACHE["nc"]
    per_core = _prep_inputs(**inputs)
    res = run_bass_kernel_spmd(nc, per_core, core_ids=list(range(NCORES)),
                               **_CACHE.get("run_kwargs", {}))
    _CACHE["last_result"] = res
    out = np.empty((B, T, V), np.float32)
    for c in range(NCORES):
        b, r = c // TP, c % TP
        out[b, :, VOC_SH * r:VOC_SH * (r + 1)] = res.results[c]["logits"]
    return out


# revision 31
# speedup vs baseline: 1.3640x; 1.1371x over previous
"""AlgebraicTransformerLM on 8 trn2 NeuronCores (Bass/Tile), v2.

Sharding: DP=2 over batch x TP=4 over heads / d_ffn / vocab (cores 0-3 =
batch 0, 4-7 = batch 1). vs v1:
  - bf16 weights + bf16 normed activations (xn) for all projections; the
    ALiBi-augmented score matmuls stay f32r (iota rows need f32 range).
  - The sublayer AllReduce is chunked by 512-token strip and fired as soon
    as that strip's output projection lands, so the collective flies while
    the other strip / next sublayer computes (keeps the PE HAM-warm).
  - Rational-sigmoid pipeline: one PSUM read (ACT copy -> bf16), fused
    |s|+1 via tensor_scalar(abs_max,add), divide (or recip+mul fallback),
    squares split ACT/DVE, causal mask via affine_select on POOL.
  - Residual adds / xn scaling split across DVE and POOL.
Host prep: fold norm weights into following matmuls, precompute xn0 (norm
of embedding output) so layer 0 starts without a device norm.
"""
import contextlib
import math

import numpy as np

import concourse.bacc as bacc
import concourse.mybir as mybir
import concourse.tile as tile
from concourse.bass_utils import run_bass_kernel_spmd

F32 = mybir.dt.float32
F32R = mybir.dt.float32r
FP16 = mybir.dt.float16

B, T, V, D, H, L = 2, 1024, 32000, 1024, 16, 4
DFF = 2730
DH = D // H
SCALE = 1.0 / math.sqrt(DH)
EPS = 1e-6

NCORES = 8
TP = 4
HPC = H // TP               # heads per core (4)
FSH = 2 * DH * HPC          # q+k rows per core (512)
VSH = DH * HPC              # v rows per core (256)
DFF_SH = 768                # padded DFF shard (4*768 >= 2730)
NFT_FF = DFF_SH // 128      # 6
VOC_SH = V // TP            # vocab shard per core (8000)
DT = D // 128               # 8
NSTRIP = T // 512           # 2
RG = [[0, 1, 2, 3], [4, 5, 6, 7]]
ALIBI = [2.0 ** (-8.0 * (i + 1) / H) for i in range(H)]

_CACHE = {}

AF = mybir.ActivationFunctionType
ALU = mybir.AluOpType


class PoolSet:
    """Route scratch tags to per-bufs pools."""

    def __init__(self, **pools):
        self._map = pools

    _n = 0

    def tile(self, shape, dtype, tag):
        PoolSet._n += 1
        return self._map[tag].tile(shape, dtype, tag=tag, name=f"{tag}_{PoolSet._n}")


def _causal_tk(s):
    return list(range((s + 1) * (512 // 128)))


def _mask_base(tk, s):
    """affine_select base for tile (tk, strip s): keep where f + base - p >= 0,
    i.e. tq >= tk. None if the whole tile is causal-valid."""
    base = s * 512 - tk * 128
    return base if tk * 128 + 127 > s * 512 else None


def _pin_act_table(arch):
    """Make every activation resolve to natural_log_exp_and_others so the
    ACT table is loaded once instead of ping-ponging between sets (each
    switch costs ~1.3us). Mutates the cached table dict in place; set ids
    keep their original indices so the walrus-side mapping is unchanged."""
    from concourse.hw_specs import get_activation_tables

    tabs = get_activation_tables(arch)
    keep = "natural_log_exp_and_others"
    mine = {AF.Abs, AF.Copy, AF.Square, AF.Exp, AF.Ln}
    if keep not in tabs or not (mine <= tabs[keep]):
        return
    for name, funcs in tabs.items():
        if name != keep:
            funcs -= mine


def build_nc(use_divide=True, use_accum_dma=True):
    nc = bacc.Bacc("TRN2", target_bir_lowering=False)
    _pin_act_table(nc.m.arch)

    x0T = nc.dram_tensor("x0T", [D, T], F32, kind="ExternalInput")
    xn0T = nc.dram_tensor("xn0T", [D, T], FP16, kind="ExternalInput")
    qaug = nc.dram_tensor("qaug", [HPC, 2, T], F32, kind="ExternalInput")
    kaug = nc.dram_tensor("kaug", [HPC, 2, T], F32, kind="ExternalInput")
    wqkT = nc.dram_tensor("wqkT", [L, D, FSH], FP16, kind="ExternalInput")
    wvT = nc.dram_tensor("wvT", [L, D, VSH], FP16, kind="ExternalInput")
    woT = nc.dram_tensor("woT", [L, VSH, D], FP16, kind="ExternalInput")
    wmT = nc.dram_tensor("wmT", [L, D, 2 * DFF_SH], FP16, kind="ExternalInput")
    w3T = nc.dram_tensor("w3T", [L, DFF_SH, D], FP16, kind="ExternalInput")
    membT = nc.dram_tensor("membT", [D, VOC_SH], FP16, kind="ExternalInput")
    logits = nc.dram_tensor("logits", [T, VOC_SH], F32, kind="ExternalOutput")
    NCH = 2 * L * NSTRIP
    cc_in = [nc.dram_tensor(f"cc_in{i}", [D, 512], FP16) for i in range(NCH)]
    cc_out = [nc.dram_tensor(f"cc_out{i}", [D, 512], FP16) for i in range(NCH)]

    with tile.TileContext(nc) as tc, contextlib.ExitStack() as ctx:
        persist = ctx.enter_context(tc.tile_pool(name="persist", bufs=1))
        psA = ctx.enter_context(tc.tile_pool(name="psA", bufs=4, space="PSUM"))
        psAcc = ctx.enter_context(tc.tile_pool(name="psAcc", bufs=2, space="PSUM"))
        psSm = ctx.enter_context(tc.tile_pool(name="psSm", bufs=1, space="PSUM"))

        x = persist.tile([128, DT, T], F32, tag="x")
        nc.sync.dma_start(x[:], x0T[:].rearrange("(dt p) t -> p dt t", p=128))
        xn = persist.tile([128, DT, T], FP16, tag="xn")
        nc.sync.dma_start(xn[:], xn0T[:].rearrange("(dt p) t -> p dt t", p=128))

        ocf = persist.tile([128, 1], F32, tag="ones_colf")
        nc.vector.memset(ocf[:], 1.0)
        ones_colb = persist.tile([128, 1], FP16, tag="ones_colb")
        nc.vector.tensor_copy(ones_colb[:], ocf[:])
        orf = persist.tile([1, 128], F32, tag="ones_rowf")
        nc.vector.memset(orf[:], 1.0)
        ones_rowb = persist.tile([1, 128], FP16, tag="ones_rowb")
        nc.vector.tensor_copy(ones_rowb[:], orf[:])
        ones_b = persist.tile([128, 1], F32, tag="ones_bias")
        nc.vector.memset(ones_b[:], 1.0)

        def sigrecip(spool, src_ps, on_act):
            """r = 1/(1+|s|) from a [128, 512] f32 PSUM tile. ACT variant:
            exp(-ln(1+|s|)) via LUT; DVE variant: reciprocal_approx_fast."""
            if on_act:
                a = spool.tile([128, 512], FP16, tag="aa")
                nc.scalar.activation(a[:], src_ps[:], AF.Abs, scale=1.0)
                ln = spool.tile([128, 512], FP16, tag="rr")
                nc.scalar.activation(ln[:], a[:], AF.Ln, bias=1.0, scale=1.0)
                r = spool.tile([128, 512], FP16, tag="aa")
                nc.scalar.activation(r[:], ln[:], AF.Exp, scale=-1.0)
            else:
                a = spool.tile([128, 512], F32, tag="af")
                nc.scalar.activation(a[:], src_ps[:], AF.Abs, scale=1.0)
                d = spool.tile([128, 512], F32, tag="af")
                nc.vector.tensor_scalar(d[:], a[:], scalar1=1.0, scalar2=None,
                                        op0=ALU.add, op1=ALU.bypass)
                r = spool.tile([128, 512], F32, tag="af")
                nc.vector.reciprocal_approx_fast(r[:], d[:])
            return r

        def sigpipe(spool, s_ps, w4_out, mb, on_act):
            """w4 = (1 + s/(1+|s|))^4 in bf16 from score psum [128, 512]."""
            r = sigrecip(spool, s_ps, on_act)
            u = spool.tile([128, 512], FP16, tag="uu")
            nc.vector.tensor_tensor(u[:], s_ps[:], r[:], ALU.mult)
            w2 = spool.tile([128, 512], FP16, tag="w2")
            nc.scalar.activation(w2[:], u[:], AF.Square, bias=ones_b[:],
                                 scale=1.0)
            if mb is not None:
                w2m = spool.tile([128, 512], FP16, tag="w2m")
                nc.gpsimd.affine_select(w2m[:], w2[:], pattern=[[1, 512]],
                                        base=mb, channel_multiplier=-1,
                                        compare_op=ALU.is_ge, fill=0.0)
                w2 = w2m
            nc.vector.tensor_tensor(w4_out[:], w2[:], w2[:], ALU.mult)

        def addnorm(s, idx, spool, dpool):
            """x[:, :, strip] += AR-chunk idx; xn strip = normed x (bf16)."""
            sl = slice(s * 512, (s + 1) * 512)
            if use_accum_dma:
                nc.gpsimd.dma_start(
                    x[:, :, sl],
                    cc_out[idx][:].rearrange("(dt p) t -> p dt t", p=128),
                    accum_op=ALU.add)
            else:
                db = dpool.tile([128, DT, 512], FP16, tag="db")
                nc.sync.dma_start(db[:],
                                  cc_out[idx][:].rearrange(
                                      "(dt p) t -> p dt t", p=128))
                for dt in range(DT):
                    eng = nc.vector if dt < 4 else nc.gpsimd
                    eng.tensor_tensor(x[:, dt, sl], x[:, dt, sl], db[:, dt],
                                      ALU.add)
            xa = spool.tile([128, DT, 512], FP16, tag="nabs")
            nc.scalar.activation(xa[:], x[:, :, sl], AF.Abs, scale=1.0)
            mag = psSm.tile([1, 512], F32, tag="small", name=f"mag{idx}")
            for dt in range(DT):
                nc.tensor.matmul(mag[:], ones_colb[:], xa[:, dt],
                                 start=(dt == 0), stop=(dt == DT - 1),
                                 skip_group_check=True)
            md = spool.tile([1, 512], F32, tag="row")
            nc.vector.tensor_scalar(md[:], mag[:], scalar1=1.0 / D,
                                    scalar2=EPS, op0=ALU.mult, op1=ALU.add)
            mr = spool.tile([1, 512], F32, tag="row")
            nc.vector.reciprocal_approx_fast(mr[:], md[:])
            mrb = spool.tile([1, 512], FP16, tag="rowb")
            nc.vector.tensor_copy(mrb[:], mr[:])
            rep = psA.tile([128, 512], F32, tag="ps", name=f"rep{idx}")
            nc.tensor.matmul(rep[:], ones_rowb[:], mrb[:], start=True,
                             stop=True)
            for dt in range(DT):
                nc.vector.tensor_tensor(xn[:, dt, sl], x[:, dt, sl], rep[:],
                                        ALU.mult)

        def attention(l, wpool, apool, dpool, spool, pending):
            # whole-layer weight loads (bf16)
            wqk = wpool.tile([128, DT, FSH], FP16, tag="wqk")
            nc.sync.dma_start(wqk[:],
                              wqkT[l].rearrange("(dt p) f -> p dt f", p=128))
            wv = wpool.tile([128, DT, VSH], FP16, tag="wv")
            nc.sync.dma_start(wv[:],
                              wvT[l].rearrange("(dt p) f -> p dt f", p=128))
            wo = wpool.tile([128, 2, D], FP16, tag="wo")
            nc.sync.dma_start(wo[:],
                              woT[l].rearrange("(pp p) f -> p pp f", p=128))

            vaug = apool.tile([128, DT, HPC * 65], FP16, tag="vaug",
                              name=f"vaug{l}")
            qa = [apool.tile([66, T], F32R, tag=f"qa{h}", name=f"qa{h}_{l}")
                  for h in range(HPC)]
            ka = [apool.tile([66, T], F32R, tag=f"ka{h}", name=f"ka{h}_{l}")
                  for h in range(HPC)]
            for h in range(HPC):
                nc.sync.dma_start(qa[h][64:66, :], qaug[h].bitcast(F32R))
                nc.sync.dma_start(ka[h][64:66, :], kaug[h].bitcast(F32R))
            asb = apool.tile([128, 2, T], FP16, tag="asb", name=f"asb{l}")

            def vproj(s):
                for tt in range(s * 4, s * 4 + 4):
                    ps = psA.tile([128, 512], F32, tag="ps",
                                  name=f"vps{l}_{tt}")
                    for dt in range(DT):
                        nc.tensor.matmul(ps[:, 0:VSH],
                                         xn[:, dt, tt * 128:(tt + 1) * 128],
                                         wv[:, dt], start=(dt == 0),
                                         stop=(dt == DT - 1))
                    for h in range(HPC):
                        nc.vector.tensor_copy(
                            vaug[:, tt, h * 65:h * 65 + 64],
                            ps[:, h * 64:(h + 1) * 64])
                        nc.vector.memset(vaug[:, tt, h * 65 + 64:h * 65 + 65],
                                         1.0)

            def qkproj(s):
                sl = slice(s * 512, (s + 1) * 512)
                for ft in range(4):
                    qk, pair = ft // 2, ft % 2
                    ps = psA.tile([128, 512], F32, tag="ps",
                                  name=f"qkps{l}_{ft}_{s}")
                    for dt in range(DT):
                        nc.tensor.matmul(ps[:],
                                         wqk[:, dt, ft * 128:(ft + 1) * 128],
                                         xn[:, dt, sl], start=(dt == 0),
                                         stop=(dt == DT - 1))
                    tgt = qa if qk == 0 else ka
                    nc.scalar.activation(tgt[2 * pair][0:64, sl], ps[0:64, :],
                                         AF.Copy, scale=1.0)
                    nc.scalar.activation(tgt[2 * pair + 1][0:64, sl],
                                         ps[64:128, :], AF.Copy, scale=1.0)

            def head_scores(h, s):
                av = psAcc.tile([65, 512], F32, tag="av", name=f"av{l}_{h}_{s}")
                tks = _causal_tk(s)
                for i, tk in enumerate(tks):
                    sc = psA.tile([128, 512], F32, tag="ps",
                                  name=f"sc{l}_{h}_{s}_{tk}")
                    nc.tensor.matmul(sc[:], ka[h][:, tk * 128:(tk + 1) * 128],
                                     qa[h][:, s * 512:(s + 1) * 512],
                                     start=True, stop=True)
                    w4 = spool.tile([128, 512], FP16, tag="w4")
                    sigpipe(spool, sc, w4, _mask_base(tk, s),
                            on_act=(i % 2 == 0))
                    nc.tensor.matmul(av[:], vaug[:, tk, h * 65:(h + 1) * 65],
                                     w4[:], start=(i == 0),
                                     stop=(i == len(tks) - 1),
                                     skip_group_check=True)
                dd = spool.tile([1, 512], F32, tag="row")
                nc.vector.tensor_scalar(dd[:], av[64:65, :],
                                        scalar1=16.0 * EPS, scalar2=None,
                                        op0=ALU.add, op1=ALU.bypass)
                dr = spool.tile([1, 512], F32, tag="row")
                nc.vector.reciprocal_approx_fast(dr[:], dd[:])
                drb = spool.tile([1, 512], FP16, tag="rowb")
                nc.vector.tensor_copy(drb[:], dr[:])
                rep = psSm.tile([64, 512], F32, tag="rep64",
                                name=f"rep{l}_{h}_{s}")
                nc.tensor.matmul(rep[:], ones_rowb[:, 0:64], drb[:],
                                 start=True, stop=True)
                reps = spool.tile([64, 512], FP16, tag="repsb")
                nc.scalar.activation(reps[:], rep[:], AF.Copy, scale=1.0)
                pair, half = h // 2, h % 2
                nc.vector.tensor_tensor(
                    asb[64 * half:64 * (half + 1), pair,
                        s * 512:(s + 1) * 512],
                    av[0:64, :], reps[:], ALU.mult)

            def outproj(s):
                sl = slice(s * 512, (s + 1) * 512)
                dls = dpool.tile([128, DT, 512], FP16, tag="dls",
                                 name=f"dla{l}_{s}")
                for ot in range(DT):
                    ps = psA.tile([128, 512], F32, tag="ps")
                    for p in range(2):
                        nc.tensor.matmul(ps[:],
                                         wo[:, p, ot * 128:(ot + 1) * 128],
                                         asb[:, p, sl], start=(p == 0),
                                         stop=(p == 1))
                    if ot % 2 == 0:
                        nc.scalar.activation(dls[:, ot], ps[:], AF.Copy,
                                             scale=1.0)
                    else:
                        nc.vector.tensor_copy(dls[:, ot], ps[:])
                return dls

            def fire(dls, idx):
                nc.sync.dma_start(
                    cc_in[idx][:].rearrange("(dt p) t -> p dt t", p=128),
                    dls[:])
                nc.gpsimd.collective_compute(
                    "AllReduce", ALU.add, ins=[cc_in[idx][:]],
                    outs=[cc_out[idx][:]], replica_groups=RG)

            idx0 = (2 * l) * NSTRIP
            # strip 0
            vproj(0)
            qkproj(0)
            for h in range(HPC):
                head_scores(h, 0)
            dls = outproj(0)
            fire(dls, idx0)
            # deferred addnorm from previous sublayer (strip 1)
            for fn in pending:
                fn()
            # strip 1
            vproj(1)
            qkproj(1)
            for h in range(HPC):
                head_scores(h, 1)
            dls = outproj(1)
            fire(dls, idx0 + 1)

        def swiglu(l, wpool, wspool, apool, dpool, spool, pending):
            w3sb = wpool.tile([128, NFT_FF, D], FP16, tag="w3")
            nc.sync.dma_start(w3sb[:],
                              w3T[l].rearrange("(ft p) f -> p ft f", p=128))

            def strip(s, idx):
                hsb = dpool.tile([128, NFT_FF, 512], FP16, tag="hsb",
                                 name=f"hsb{l}_{s}")
                sl = slice(s * 512, (s + 1) * 512)
                for ft in range(NFT_FF):
                    wmg = wspool.tile([128, DT, 128], FP16, tag="wmg")
                    nc.sync.dma_start(
                        wmg[:], wmT[l][:, ft * 128:(ft + 1) * 128]
                        .rearrange("(dt p) f -> p dt f", p=128))
                    wmv = wspool.tile([128, DT, 128], FP16, tag="wmv")
                    nc.sync.dma_start(
                        wmv[:],
                        wmT[l][:, DFF_SH + ft * 128:DFF_SH + (ft + 1) * 128]
                        .rearrange("(dt p) f -> p dt f", p=128))
                    gps = psA.tile([128, 512], F32, tag="ps")
                    vps = psA.tile([128, 512], F32, tag="ps")
                    for dt in range(DT):
                        nc.tensor.matmul(gps[:], wmg[:, dt],
                                         xn[:, dt, sl], start=(dt == 0),
                                         stop=(dt == DT - 1))
                    for dt in range(DT):
                        nc.tensor.matmul(vps[:], wmv[:, dt],
                                         xn[:, dt, sl], start=(dt == 0),
                                         stop=(dt == DT - 1))
                    # h = g*(1+u)*v with u = g/(1+|g|)  (x0.5 folded into w3)
                    r = sigrecip(spool, gps, on_act=True)
                    gb = spool.tile([128, 512], FP16, tag="gb")
                    nc.vector.tensor_copy(gb[:], gps[:])
                    u = spool.tile([128, 512], FP16, tag="uu")
                    nc.vector.tensor_tensor(u[:], gb[:], r[:], ALU.mult)
                    t = spool.tile([128, 512], FP16, tag="tt")
                    nc.scalar.activation(t[:], u[:], AF.Copy, bias=1.0,
                                         scale=1.0)
                    m2 = spool.tile([128, 512], FP16, tag="m2")
                    nc.gpsimd.tensor_tensor(m2[:], gb[:], t[:], ALU.mult)
                    nc.vector.tensor_tensor(hsb[:, ft], m2[:], vps[:],
                                            ALU.mult)
                dls = dpool.tile([128, DT, 512], FP16, tag="dls",
                                 name=f"dlm{l}_{s}")
                for ot in range(DT):
                    ps = psA.tile([128, 512], F32, tag="ps")
                    for ft in range(NFT_FF):
                        nc.tensor.matmul(ps[:],
                                         w3sb[:, ft, ot * 128:(ot + 1) * 128],
                                         hsb[:, ft], start=(ft == 0),
                                         stop=(ft == NFT_FF - 1))
                    if ot % 2 == 0:
                        nc.scalar.activation(dls[:, ot], ps[:], AF.Copy,
                                             scale=1.0)
                    else:
                        nc.vector.tensor_copy(dls[:, ot], ps[:])
                nc.sync.dma_start(
                    cc_in[idx][:].rearrange("(dt p) t -> p dt t", p=128),
                    dls[:])
                nc.gpsimd.collective_compute(
                    "AllReduce", ALU.add, ins=[cc_in[idx][:]],
                    outs=[cc_out[idx][:]], replica_groups=RG)

            idx0 = (2 * l + 1) * NSTRIP
            strip(0, idx0)
            for fn in pending:
                fn()
            strip(1, idx0 + 1)

        with tc.tile_pool(name="wpool", bufs=1) as wpool, \
             tc.tile_pool(name="wspool", bufs=3) as wspool, \
             tc.tile_pool(name="apool", bufs=1) as apool, \
             tc.tile_pool(name="dpool", bufs=2) as dpool, \
             tc.tile_pool(name="scrpool", bufs=3) as scrpool, \
             tc.tile_pool(name="ewpool", bufs=2) as ewpool, \
             tc.tile_pool(name="w4pool", bufs=2) as w4pool, \
             tc.tile_pool(name="npool", bufs=1) as npool, \
             tc.tile_pool(name="mpool", bufs=2) as mpool, \
             tc.tile_pool(name="rowpool", bufs=2) as rowpool:
            spool = PoolSet(aa=scrpool, rr=scrpool, uu=scrpool, af=scrpool,
                            w2=scrpool, w2m=scrpool, gb=ewpool, tt=ewpool,
                            m2=ewpool, w4=w4pool, nabs=npool, repb=mpool,
                            repsb=mpool, row=rowpool, rowb=rowpool)

            def an(s, idx):
                return lambda: addnorm(s, idx, spool, dpool)

            # AR chunk ids per layer: 4l (att s0), 4l+1 (att s1),
            # 4l+2 (ffn s0), 4l+3 (ffn s1). Each addnorm is emitted as late
            # as possible so the collective flies under compute: att-s1
            # addnorm lands between the ffn strips, ffn-s1 addnorm between
            # the next layer's attention strips.
            pend = []
            for l in range(L):
                attention(l, wpool, apool, dpool, spool, pend)
                addnorm(0, 4 * l, spool, dpool)
                swiglu(l, wpool, wspool, apool, dpool, spool,
                       [an(1, 4 * l + 1)])
                addnorm(0, 4 * l + 2, spool, dpool)
                pend = [an(1, 4 * l + 3)]
            for fn in pend:
                fn()

        with tc.tile_pool(name="lmw", bufs=2) as lmw, \
             tc.tile_pool(name="lms", bufs=4) as lms:
            nvs = (VOC_SH + 511) // 512
            for vs in range(nvs):
                vw = min(512, VOC_SH - vs * 512)
                wt = lmw.tile([128, DT, 512], FP16, tag="wemb")
                nc.sync.dma_start(
                    wt[:, :, :vw], membT[:, vs * 512:vs * 512 + vw]
                    .rearrange("(dt p) f -> p dt f", p=128))
                for tt in range(DT):
                    ps = psA.tile([128, 512], F32, tag="ps")
                    for dt in range(DT):
                        nc.tensor.matmul(ps[:, :vw],
                                         xn[:, dt, tt * 128:(tt + 1) * 128],
                                         wt[:, dt, :vw],
                                         start=(dt == 0), stop=(dt == DT - 1))
                    ls = lms.tile([128, 512], F32, tag="lmsb")
                    if tt % 2 == 0:
                        nc.scalar.activation(ls[:, :vw], ps[:, :vw], AF.Copy,
                                             scale=1.0)
                    else:
                        nc.vector.tensor_copy(ls[:, :vw], ps[:, :vw])
                    nc.sync.dma_start(
                        logits[tt * 128:(tt + 1) * 128,
                               vs * 512:vs * 512 + vw],
                        ls[:, :vw])
    nc.compile()
    return nc


def _prep_inputs(input_ids, emb, qkv_w, out_w, n1_w, n2_w, wm_w, w3_w, fn_w):
    ids = np.asarray(input_ids)
    emb = np.asarray(emb, dtype=np.float32)
    x0 = emb[ids]                                   # [B, T, D]
    mag = np.mean(np.abs(x0), axis=-1, keepdims=True)
    xn0 = x0 / (mag + EPS)
    iota = np.arange(T, dtype=np.float32)
    qkv_w = np.asarray(qkv_w, dtype=np.float32)
    out_w = np.asarray(out_w, dtype=np.float32)
    wm_w = np.asarray(wm_w, dtype=np.float32)
    w3_w = np.asarray(w3_w, dtype=np.float32)
    n1_w = np.asarray(n1_w, dtype=np.float32)
    n2_w = np.asarray(n2_w, dtype=np.float32)
    fn_w = np.asarray(fn_w, dtype=np.float32)
    per_core = []
    for c in range(NCORES):
        b, r = c // TP, c % TP
        heads = list(range(HPC * r, HPC * r + HPC))
        qa = np.stack([np.stack([-iota, np.full(T, ALIBI[h], np.float32)])
                       for h in heads]).astype(np.float32)
        ka = np.stack([np.stack([np.full(T, ALIBI[h], np.float32), iota])
                       for h in heads]).astype(np.float32)
        wqk = np.empty((L, D, FSH), np.float32)
        wv = np.empty((L, D, VSH), np.float32)
        wo = np.empty((L, VSH, D), np.float32)
        wm = np.zeros((L, D, 2 * DFF_SH), np.float32)
        w3 = np.zeros((L, DFF_SH, D), np.float32)
        for l in range(L):
            q3 = qkv_w[l].reshape(3, H, DH, D)
            qrows = q3[0, heads].reshape(VSH, D) * SCALE
            krows = q3[1, heads].reshape(VSH, D)
            vrows = q3[2, heads].reshape(VSH, D)
            n1 = n1_w[l][:, None]                   # fold into d-rows of W^T
            wqk[l] = np.concatenate([qrows, krows], 0).T * n1
            wv[l] = vrows.T * n1
            ow = out_w[l].reshape(D, H, DH)[:, heads].reshape(D, VSH)
            wo[l] = ow.T
            n2 = n2_w[l][:, None]
            g0, g1 = DFF_SH * r, min(DFF_SH * (r + 1), DFF)
            ng = g1 - g0
            if ng > 0:
                wm[l, :, :ng] = wm_w[l][g0:g1].T * n2
                wm[l, :, DFF_SH:DFF_SH + ng] = wm_w[l][DFF + g0:DFF + g1].T * n2
                w3[l, :ng] = 0.5 * w3_w[l][:, g0:g1].T
        memb = (emb[VOC_SH * r:VOC_SH * (r + 1)] * fn_w[None, :]).T
        per_core.append(dict(
            x0T=np.ascontiguousarray(x0[b].T),
            xn0T=np.ascontiguousarray(xn0[b].T).astype(np.float16),
            qaug=qa, kaug=ka,
            wqkT=np.ascontiguousarray(wqk).astype(np.float16),
            wvT=np.ascontiguousarray(wv).astype(np.float16),
            woT=np.ascontiguousarray(wo).astype(np.float16),
            wmT=np.ascontiguousarray(wm).astype(np.float16),
            w3T=np.ascontiguousarray(w3).astype(np.float16),
            membT=np.ascontiguousarray(memb).astype(np.float16),
        ))
    return per_core


def kernel(**inputs):
    if "nc" not in _CACHE:
        try:
            _CACHE["nc"] = build_nc(use_divide=True)
        except Exception:
            _CACHE["nc"] = build_nc(use_divide=False)
    nc = _CACHE["nc"]
    per_core = _prep_inputs(**inputs)
    res = run_bass_kernel_spmd(nc, per_core, core_ids=list(range(NCORES)),
                               **_CACHE.get("run_kwargs", {}))
    _CACHE["last_result"] = res
    out = np.empty((B, T, V), np.float32)
    for c in range(NCORES):
        b, r = c // TP, c % TP
        out[b, :, VOC_SH * r:VOC_SH * (r + 1)] = res.results[c]["logits"]
    return out
